# revision 25
# baseline (speedup 1.0000x reference)
"""Trainium2 Bass kernel for nn_KStackModel (sparse_attention).

Strategy: data-parallel over batch (8 batches -> 8 cores, no collectives).
Heavy matmuls run bf16 (1 cyc/row on the PE vs 4 for fp32); the MLP up/down
matmuls run fp8e4 in DoubleRow perf mode (0.5 cyc/row, K=256/instr).

Per core (feature-major activations, tokens on the free axis):

  h ships twice, pre-swizzled on the host into single-DMA layouts:
  h_tok [t,d] bf16 (2 halves) and hTb [d,t] bf16. DMA dispatch is ~650ns
  each on the sync queue, so everything ships in ~10 large transfers.
  rms1: ACT Square+accum_out on h_tok -> rstd_col [128,8], in two halves so
  the base matmuls start after the first 4 token tiles; hn = h_tok *
  rstd_col in place (DVE per-partition scale). rstd_row via PE transposes.
  xv^T[r,t] = v_eff.T @ hTb, scaled by rstd_row after (commutes);
  mixed^T = DVE tensor_tensor_scan (state = gamma*state + xv).
  out^T[d,t] = sum_s hn[s,d].T @ KT[s,t] over causal 512-blocks (bf16),
  evicted to bf16 on the (otherwise idle) GPSIMD engine.
  h1^T = pw.T @ out^T + pu.T @ mixed^T + proj_b + hTb (fp32 accum; pu =
  alpha*proj_w@u folds the low-rank output through proj); rms2 stats
  interleaved (squares on GPSIMD, ones-column reduce on the PE).
  h2 = h1 * rstd2 -> fp8 pair tiles [128,2,W] (DoubleRow rhs layout).
  g8 = fp8(gelu((8*up).T @ h2 / 8 + up_b))   (weights pre-scaled x8 on the
  host to dodge fp8 subnormals; /8 exact via the ACT scale operand).
  y^T = (16*dw).T @ g8 / 16 + down_b + h1^T; DMA out fp32, host transposes.

All weight folds (norm scales, gate, alpha, proj@u, fp8 scaling) are exact
host-side algebra; weights ship bf16/fp8, cutting PCIe and HBM traffic.
"""
import numpy as np
from contextlib import ExitStack

import concourse.bass as bass
import concourse.bacc as bacc
import concourse.tile as tile
from concourse import mybir
from concourse.bass_utils import run_bass_kernel_spmd
import ml_dtypes

B, W, D, R, F = 8, 1024, 1024, 32, 2048
NT, ND, NF = W // 128, D // 128, F // 128   # 8, 8, 16
FP = mybir.dt.float32
BF = mybir.dt.bfloat16
F8 = mybir.dt.float8e4
GAMMA_MIN, GAMMA_MAX = 0.15, 1.0
AF = mybir.ActivationFunctionType
ALU = mybir.AluOpType
PM = mybir.MatmulPerfMode
BFNP = ml_dtypes.bfloat16
F8NP = ml_dtypes.float8_e4m3
UP_SCALE = 8.0
DW_SCALE = 16.0
# (sj, tcc) block order of the packed causal KT blocks
KT_BLOCKS = [(sj, 0) for sj in range(4)] + [(sj, 1) for sj in range(8)]


def _emit(ctx, tc, a):
    nc = tc.nc

    con = ctx.enter_context(tc.tile_pool(name="con", bufs=1))
    h1p = ctx.enter_context(tc.tile_pool(name="h1p", bufs=8))
    htkp = ctx.enter_context(tc.tile_pool(name="htkp", bufs=8))
    hbp = ctx.enter_context(tc.tile_pool(name="hbp", bufs=1))
    sq2p = ctx.enter_context(tc.tile_pool(name="sq2p", bufs=2))
    wp = ctx.enter_context(tc.tile_pool(name="wp", bufs=1))
    outp = ctx.enter_context(tc.tile_pool(name="outp", bufs=8))
    h28p = ctx.enter_context(tc.tile_pool(name="h28p", bufs=4))
    g8p = ctx.enter_context(tc.tile_pool(name="g8p", bufs=8))
    yst = ctx.enter_context(tc.tile_pool(name="yst", bufs=3))
    rows = ctx.enter_context(tc.tile_pool(name="rows", bufs=2))
    r32 = ctx.enter_context(tc.tile_pool(name="r32", bufs=3))
    pmm = ctx.enter_context(tc.tile_pool(name="pmm", bufs=5, space="PSUM"))
    psm = ctx.enter_context(tc.tile_pool(name="psm", bufs=1, space="PSUM"))

    # ---- DMA queue: h_tok h0, KT0, h_tok h1, consts, hTb, KT1, pw, up8, dw8 ----
    htok_t = [htkp.tile([128, D], BF, tag="htok", name=f"htok{hf}")
              for hf in range(8)]
    nc.sync.dma_start(htok_t[0][:], a["h_tok"][0, :, :])
    nc.sync.dma_start(htok_t[1][:], a["h_tok"][1, :, :])
    kt0 = con.tile([128, 4, 512], BF, tag="kt0")
    nc.sync.dma_start(kt0[:], a["KT0p"][:, :, :])
    for hf in range(2, 8):
        nc.sync.dma_start(htok_t[hf][:], a["h_tok"][hf, :, :])

    def htok(ti):
        return htok_t[ti][:]

    # packed fp32 consts: eyef | projb | downb | upb
    cpf = con.tile([128, 128 + 2 * ND + NF], FP, tag="cpf")
    nc.sync.dma_start(cpf[:], a["cpf"][:, :])
    eyef = cpf[:, 0:128]
    projb = cpf[:, 128:128 + ND]
    downb = cpf[:, 128 + ND:128 + 2 * ND]
    upb = cpf[:, 128 + 2 * ND:128 + 2 * ND + NF]
    cpb = con.tile([128, ND * R], BF, tag="cpb")
    nc.sync.dma_start(cpb[:], a["cpb"][:, :])

    def v_sb(dj):
        return cpb[:, dj * R:(dj + 1) * R]

    gam_c = con.tile([R, 1], FP, tag="gam_c")
    nc.sync.dma_start(gam_c[:], a["gamma_t"][:, :])
    pu_sb = con.tile([R, D], BF, tag="pu_sb")
    nc.sync.dma_start(pu_sb[:], a["puT"][:, :])

    kt1 = con.tile([128, 8, 512], BF, tag="kt1")
    nc.sync.dma_start(kt1[:], a["KT1p"][:, :, :])

    # hTb [128, 8, W] bf16 (feature-major h: xv moving operand + residual)
    hbt = hbp.tile([128, ND, W], BF, tag="hb")
    nc.sync.dma_start(hbt[:], a["hTb"][:, :, :])

    def hb(dj):
        return hbt[:, dj, :]

    def kts(sj, tcc):
        return kt0[:, sj, :] if tcc == 0 else kt1[:, sj, :]

    pw_t = wp.tile([128, ND, D], BF, tag="pw")
    nc.sync.dma_start(pw_t[:], a["pw"][:, :, :])
    up_t = wp.tile([128, NF, 4, 2, 128], F8, tag="up8")
    nc.sync.dma_start(up_t[:], a["up8"][:, :, :, :, :])
    dw_t = wp.tile([128, ND, 8, 2, 128], F8, tag="dw8")
    nc.sync.dma_start(dw_t[:], a["dw8"][:, :, :, :, :])

    # ---- const-ap registrations (memsets, no DMA) ----
    zeros_c = con.tile([128, 1], FP, tag="zeros_c")
    nc.vector.memset(zeros_c[:], 0.0)
    nc.const_aps.aps[(FP, 0.0)] = zeros_c[:]
    eps_c = con.tile([128, 1], FP, tag="eps_c")
    nc.vector.memset(eps_c[:], 1e-8)
    nc.const_aps.aps[(FP, 1e-8)] = eps_c[:]
    ones_cf = con.tile([128, 1], FP, tag="ones_cf")
    nc.vector.memset(ones_cf[:], 1.0)
    ones_r128 = con.tile([1, 128], FP, tag="ones_r128")
    nc.vector.memset(ones_r128[:], 1.0)
    acc2 = con.tile([128, W], BF, tag="acc2")
    ones_rb = con.tile([1, 128], BF, tag="ones_rb")
    nc.vector.memset(ones_rb[:], 1.0)
    ones_cb = con.tile([128, 1], BF, tag="ones_cb")
    nc.vector.memset(ones_cb[:], 1.0)
    # pre-warm the ACT function tables while the first DMAs stream
    # (scratch target: std_col[:, 0:1] is overwritten later by the real Sqrt)

    # ---- rms1 stats (token-major ACT accum), two halves; hn in place ----
    ssq_col = con.tile([128, NT], FP, tag="ssq_col")
    std_col = con.tile([128, NT], FP, tag="std_col")
    rstd_col = con.tile([128, NT], FP, tag="rstd_col")
    nc.scalar.activation(std_col[:, 0:1], zeros_c[:], AF.Square)
    nc.scalar.activation(std_col[:, 0:1], zeros_c[:], AF.Sqrt, bias=1e-8, scale=1.0)
    nc.scalar.activation(std_col[:, 0:1], zeros_c[:], AF.Identity,
                         bias=eps_c[:, 0:1], scale=1.0)
    for ti in range(NT):
        nc.scalar.activation(acc2[:], htok(ti), AF.Square,
                             accum_out=ssq_col[:, ti:ti + 1])
        nc.scalar.activation(std_col[:, ti:ti + 1], ssq_col[:, ti:ti + 1],
                             AF.Sqrt, bias=1e-8, scale=1.0 / D)
        nc.vector.reciprocal(rstd_col[:, ti:ti + 1], std_col[:, ti:ti + 1])
        nc.vector.tensor_scalar_mul(htok(ti), htok(ti), rstd_col[:, ti:ti + 1])

    nc.vector.memset(acc2[:], 0.0)

    # ---- base mixing: out^T[d,t] = sum_s hn[s,d].T @ KT[s,t] ----
    outT = [outp.tile([128, W], BF, tag="outT", name=f"outT{dj}")
            for dj in range(ND)]

    def base_piece(tcc, lo, hi, sjs):
        # columns [lo, hi) of the tcc-chunk; causal s-blocks sjs
        w = hi - lo
        for dj in range(ND):
            po = pmm.tile([128, 512], FP, tag="pmm")
            for i, sj in enumerate(sjs):
                nc.tensor.matmul(po[:, 0:w],
                                 htok(sj)[:, dj * 128:(dj + 1) * 128],
                                 kts(sj, tcc)[:, lo:hi],
                                 start=(i == 0), stop=(i == len(sjs) - 1))
            nc.vector.tensor_copy(
                outT[dj][:, tcc * 512 + lo:tcc * 512 + hi], po[:, 0:w])

    base_piece(0, 0, 128, [0])
    base_piece(0, 128, 256, [0, 1])
    base_piece(0, 256, 512, [0, 1, 2, 3])

    # rstd_row [1, W] for the xv scale, via PE transposes of rstd_col
    prow = psm.tile([1, W], FP, tag="prow")
    for ti in range(NT):
        nc.tensor.transpose(prow[0:1, ti * 128:(ti + 1) * 128],
                            rstd_col[:, ti:ti + 1], eyef)
    rstd_row = rows.tile([1, W], FP, tag="row")
    nc.vector.tensor_copy(rstd_row[:], prow[:])

    base_piece(1, 0, 256, list(range(6)))
    base_piece(1, 256, 512, list(range(8)))

    # ---- xv^T [R, W] = v_eff.T @ h (raw), then * rstd ----
    xv_raw = r32.tile([R, W], FP, tag="r32")
    for c in range(2):
        pxv = psm.tile([R, 512], FP, tag="pxv", bufs=1)
        for dj in range(ND):
            nc.tensor.matmul(pxv[:], v_sb(dj), hb(dj)[:, c * 512:(c + 1) * 512],
                             start=(dj == 0), stop=(dj == ND - 1))
        nc.vector.tensor_copy(xv_raw[:, c * 512:(c + 1) * 512], pxv[:])
    rep32 = r32.tile([R, W], FP, tag="r32")
    for c in range(2):
        prep = psm.tile([R, 512], FP, tag="pxv", bufs=1)
        nc.tensor.matmul(prep[:], ones_r128[0:1, 0:R],
                         rstd_row[0:1, c * 512:(c + 1) * 512], start=True, stop=True)
        nc.vector.tensor_copy(rep32[:, c * 512:(c + 1) * 512], prep[:])
    xvT = r32.tile([R, W], FP, tag="r32")
    nc.vector.tensor_mul(xvT[:], xv_raw[:], rep32[:])

    # ---- decay scan (gamma broadcast along t); bf16 copy for the pu matmul ----
    mixedT = r32.tile([R, W], FP, tag="r32")
    nc.vector.tensor_tensor_scan(mixedT[:], gam_c[:].to_broadcast((R, W)), xvT[:],
                                 0.0, ALU.mult, ALU.add)
    mixedb = con.tile([R, W], BF, tag="mixedb")
    nc.vector.tensor_copy(mixedb[:], mixedT[:])

    # ---- h1^T = pw.T @ out^T + pu.T @ mixed^T + proj_b + h  (tcc-major);
    #      rms2 stats ride along: squares on GPSIMD, block-sums into acc2 (DVE),
    #      per-chunk rstd2/rep2/h28 overlap the other chunk's matmuls ----
    pssq2 = psm.tile([1, W], FP, tag="prow", bufs=1)
    h1T = [h1p.tile([128, W], FP, tag="h1", name=f"h1_{dj2}") for dj2 in range(ND)]
    h28 = [h28p.tile([128, 2, W], F8, tag="h28", name=f"h28_{m}") for m in range(4)]
    std2 = rows.tile([1, W], FP, tag="row")
    rstd2_row = rows.tile([1, W], FP, tag="row")
    rstd2b = con.tile([1, W], BF, tag="rstd2b")
    rep2 = con.tile([128, W], FP, tag="rep2")

    def rms2_reduce(c):
        sl = slice(c * 512, (c + 1) * 512)
        nc.tensor.matmul(pssq2[0:1, sl], ones_cb[:], acc2[:, sl],
                         start=True, stop=True)
        nc.scalar.activation(std2[0:1, sl], pssq2[0:1, sl], AF.Sqrt,
                             bias=1e-8, scale=1.0 / D)
        nc.vector.reciprocal(rstd2_row[0:1, sl], std2[0:1, sl])
        nc.vector.tensor_copy(rstd2b[0:1, sl], rstd2_row[0:1, sl])

    def rep2_h28(c):
        sl = slice(c * 512, (c + 1) * 512)
        pr2 = pmm.tile([128, 512], FP, tag="pmm")
        nc.tensor.matmul(pr2[:], ones_rb[:], rstd2b[0:1, sl],
                         start=True, stop=True)
        nc.vector.tensor_copy(rep2[:, sl], pr2[:])
        for m in range(4):
            for i in range(2):
                nc.vector.tensor_mul(h28[m][:, i, sl], h1T[2 * m + i][:, sl],
                                     rep2[:, sl])

    for tcc in range(2):
        sl = slice(tcc * 512, (tcc + 1) * 512)
        for dj2 in range(ND):
            if tcc == 1 and dj2 == 3:
                rms2_reduce(0)
            if tcc == 1 and dj2 == 5:
                rep2_h28(0)
            ph = pmm.tile([128, 512], FP, tag="pmm")
            for dj in range(ND):
                nc.tensor.matmul(ph[:], pw_t[:, dj2, dj * 128:(dj + 1) * 128],
                                 outT[dj][:, sl],
                                 start=(dj == 0), stop=False)
            nc.tensor.matmul(ph[:], pu_sb[:, dj2 * 128:(dj2 + 1) * 128],
                             mixedb[:, sl], start=False, stop=True)
            h1 = h1T[dj2]
            nc.scalar.activation(h1[:, sl], ph[:], AF.Identity,
                                 bias=projb[:, dj2:dj2 + 1], scale=1.0)
            nc.vector.tensor_add(h1[:, sl], h1[:, sl], hb(dj2)[:, sl])
            s2 = sq2p.tile([128, 512], BF, tag="sq2")
            nc.gpsimd.tensor_mul(s2[:], h1[:, sl], h1[:, sl])
            nc.vector.tensor_add(acc2[:, sl], acc2[:, sl], s2[:])

    # ---- up + gelu then down + residual, tcc-major (fp8 DoubleRow) ----
    g8 = [g8p.tile([128, 2, W], F8, tag="g8", name=f"g8_{m}") for m in range(NF // 2)]
    for tcc in range(2):
        sl = slice(tcc * 512, (tcc + 1) * 512)
        for fi in range(NF):
            pg = pmm.tile([128, 512], FP, tag="pmm")
            for m in range(4):
                nc.tensor.matmul(pg[:], up_t[:, fi, m, :, :], h28[m][:, :, sl],
                                 start=(m == 0), stop=(m == 3),
                                 perf_mode=PM.DoubleRow)
            nc.scalar.activation(g8[fi // 2][:, fi % 2, sl], pg[:],
                                 AF.Gelu_apprx_tanh,
                                 bias=upb[:, fi:fi + 1], scale=1.0 / UP_SCALE)
        for dj2 in range(ND):
            if tcc == 0 and dj2 == 1:
                rms2_reduce(1)
            if tcc == 0 and dj2 == 3:
                rep2_h28(1)
            py = pmm.tile([128, 512], FP, tag="pmm")
            for m in range(8):
                nc.tensor.matmul(py[:], dw_t[:, dj2, m, :, :], g8[m][:, :, sl],
                                 start=(m == 0), stop=(m == 7),
                                 perf_mode=PM.DoubleRow)
            y = yst.tile([128, 512], FP, tag="yst")
            nc.scalar.activation(y[:], py[:], AF.Identity,
                                 bias=downb[:, dj2:dj2 + 1], scale=1.0 / DW_SCALE)
            nc.vector.tensor_add(y[:], y[:], h1T[dj2][:, sl])
            nc.sync.dma_start(a["yT"][dj2, :, sl], y[:])


_NC_CACHE = {}


def _build():
    if "nc" in _NC_CACHE:
        return _NC_CACHE["nc"]
    nc = bacc.Bacc("TRN2", target_bir_lowering=False, debug=False)

    def P(name, shape, dt=FP, out=False):
        return nc.declare_dram_parameter(name, list(shape), dt, isOutput=out)

    a = dict(
        h_tok=P("h_tok", (NT, 128, D), BF),
        hTb=P("hTb", (128, ND, W), BF),
        KT0p=P("KT0p", (128, 4, 512), BF),
        KT1p=P("KT1p", (128, 8, 512), BF),
        cpf=P("cpf", (128, 128 + 2 * ND + NF)),
        cpb=P("cpb", (128, ND * R), BF),
        puT=P("puT", (R, D), BF),
        pw=P("pw", (128, ND, D), BF),
        up8=P("up8", (128, NF, 4, 2, 128), F8),
        dw8=P("dw8", (128, ND, 8, 2, 128), F8),
        gamma_t=P("gamma_t", (R, 1)),
        yT=P("yT", (ND, 128, W), out=True),
    )
    with ExitStack() as ctx:
        tcx = ctx.enter_context(tile.TileContext(nc))
        _emit(ctx, tcx, a)
    nc.finalize()
    _NC_CACHE["nc"] = nc
    return nc


def _sigmoid(x):
    return 1.0 / (1.0 + np.exp(-x))


def host_prep(inputs):
    """Exact host-side weight folds/layout. Returns the shared in_map dict."""
    f32 = np.float32
    ns1 = np.asarray(inputs["norm1_scale"], f32)
    ns2 = np.asarray(inputs["norm2_scale"], f32)
    gate = f32(_sigmoid(np.float64(np.asarray(inputs["gate_logit"]))))
    alpha = f32(_sigmoid(np.float64(np.asarray(inputs["alpha_logit"]))))
    gamma = (GAMMA_MIN + (GAMMA_MAX - GAMMA_MIN)
             * _sigmoid(np.asarray(inputs["decay_logit"], np.float64))).astype(f32)

    kb = np.asarray(inputs["k_base"], f32) * np.tril(np.ones((W, W), f32))
    KT = np.ascontiguousarray((gate * kb).T).astype(BFNP)
    KT0p = np.stack([KT[sj * 128:(sj + 1) * 128, 0:512] for sj in range(4)], axis=1)
    KT1p = np.stack([KT[sj * 128:(sj + 1) * 128, 512:1024] for sj in range(8)], axis=1)
    v_eff = (ns1[:, None] * np.asarray(inputs["v"], f32)).astype(BFNP)
    cpb = np.ascontiguousarray(
        v_eff.reshape(ND, 128, R).transpose(1, 0, 2).reshape(128, ND * R))
    proj_w = np.asarray(inputs["proj_w"], f32)
    puT = np.ascontiguousarray(
        (alpha * (proj_w @ np.asarray(inputs["u"], f32))).T).astype(BFNP)
    pw_lhsT = (proj_w * ns1[None, :]).T
    up_lhsT = (np.asarray(inputs["up_w"], f32) * ns2[None, :]).T
    dw_lhsT = np.asarray(inputs["down_w"], f32).T

    # stationary-block layouts, contraction-sub-128 on the partition axis
    pw = np.ascontiguousarray(
        pw_lhsT.reshape(ND, 128, ND, 128).transpose(1, 2, 0, 3).reshape(128, ND, D)
    ).astype(BFNP)
    up8 = np.ascontiguousarray(
        (UP_SCALE * up_lhsT).reshape(4, 2, 128, NF, 128).transpose(2, 3, 0, 1, 4)
    ).astype(F8NP)
    dw8 = np.ascontiguousarray(
        (DW_SCALE * dw_lhsT).reshape(8, 2, 128, ND, 128).transpose(2, 3, 0, 1, 4)
    ).astype(F8NP)

    cpf = np.zeros((128, 128 + 2 * ND + NF), f32)
    cpf[:, 0:128] = np.eye(128, dtype=f32)
    cpf[:, 128:128 + ND] = np.asarray(inputs["proj_b"], f32).reshape(ND, 128).T
    cpf[:, 128 + ND:128 + 2 * ND] = (
        np.asarray(inputs["down_b"], f32).reshape(ND, 128).T)
    cpf[:, 128 + 2 * ND:] = np.asarray(inputs["up_b"], f32).reshape(NF, 128).T

    return dict(
        KT0p=KT0p, KT1p=KT1p, cpf=cpf, cpb=cpb, puT=puT, pw=pw, up8=up8, dw8=dw8,
        gamma_t=np.ascontiguousarray(gamma[:, None]),
    )


def make_in_maps(inputs):
    shared = host_prep(inputs)
    h = np.asarray(inputs["h"], np.float32)
    in_maps = []
    for b in range(B):
        m = dict(shared)
        hb16 = h[b].astype(BFNP)
        m["h_tok"] = np.ascontiguousarray(hb16.reshape(NT, 128, D))
        m["hTb"] = np.ascontiguousarray(
            hb16.T.reshape(ND, 128, W).transpose(1, 0, 2))
        in_maps.append(m)
    return in_maps


def kernel(**inputs):
    nc = _build()
    in_maps = make_in_maps(inputs)
    res = run_bass_kernel_spmd(nc, in_maps, list(range(B)))
    out = np.stack([np.asarray(res.results[i]["yT"]).reshape(D, W).T
                    for i in range(B)])
    return np.ascontiguousarray(out.astype(np.float32))


# revision 27
# speedup vs baseline: 1.2344x; 1.2344x over previous
"""Trainium2 Bass kernel for nn_KStackModel (sparse_attention).

Strategy: data-parallel over batch (8 batches -> 8 cores, no collectives).
Heavy matmuls run bf16 (1 cyc/row on the PE vs 4 for fp32); the MLP up/down
matmuls run fp8e4 in DoubleRow perf mode (0.5 cyc/row, K=256/instr).

Per core (feature-major activations, tokens on the free axis):

  h ships twice, pre-swizzled on the host into single-DMA layouts:
  h_tok [t,d] bf16 (2 halves) and hTb [d,t] bf16. DMA dispatch is ~650ns
  each on the sync queue, so everything ships in ~10 large transfers.
  rms1: ACT Square+accum_out on h_tok -> rstd_col [128,8], in two halves so
  the base matmuls start after the first 4 token tiles; hn = h_tok *
  rstd_col in place (DVE per-partition scale). rstd_row via PE transposes.
  xv^T[r,t] = v_eff.T @ hTb, scaled by rstd_row after (commutes);
  mixed^T = DVE tensor_tensor_scan (state = gamma*state + xv).
  out^T[d,t] = sum_s hn[s,d].T @ KT[s,t] over causal 512-blocks (bf16),
  evicted to bf16 on the (otherwise idle) GPSIMD engine.
  h1^T = pw.T @ out^T + pu.T @ mixed^T + proj_b + hTb (fp32 accum; pu =
  alpha*proj_w@u folds the low-rank output through proj); rms2 stats
  interleaved (squares on GPSIMD, ones-column reduce on the PE).
  h2 = h1 * rstd2 -> fp8 pair tiles [128,2,W] (DoubleRow rhs layout).
  g8 = fp8(gelu((8*up).T @ h2 / 8 + up_b))   (weights pre-scaled x8 on the
  host to dodge fp8 subnormals; /8 exact via the ACT scale operand).
  y^T = (16*dw).T @ g8 / 16 + down_b + h1^T; DMA out fp32, host transposes.

All weight folds (norm scales, gate, alpha, proj@u, fp8 scaling) are exact
host-side algebra; weights ship bf16/fp8, cutting PCIe and HBM traffic.
"""
import numpy as np
from contextlib import ExitStack

import concourse.bass as bass
import concourse.bacc as bacc
import concourse.tile as tile
from concourse import mybir
from concourse.bass_utils import run_bass_kernel_spmd
import ml_dtypes

B, W, D, R, F = 8, 1024, 1024, 32, 2048
NT, ND, NF = W // 128, D // 128, F // 128   # 8, 8, 16
FP = mybir.dt.float32
BF = mybir.dt.bfloat16
F8 = mybir.dt.float8e4
GAMMA_MIN, GAMMA_MAX = 0.15, 1.0
AF = mybir.ActivationFunctionType
ALU = mybir.AluOpType
PM = mybir.MatmulPerfMode
BFNP = ml_dtypes.bfloat16
F8NP = ml_dtypes.float8_e4m3
UP_SCALE = 8.0
DW_SCALE = 16.0
# (sj, tcc) block order of the packed causal KT blocks
KT_BLOCKS = [(sj, 0) for sj in range(4)] + [(sj, 1) for sj in range(8)]


def _emit(ctx, tc, a):
    nc = tc.nc

    con = ctx.enter_context(tc.tile_pool(name="con", bufs=1))
    h1p = ctx.enter_context(tc.tile_pool(name="h1p", bufs=8))
    htkp = ctx.enter_context(tc.tile_pool(name="htkp", bufs=8))
    hbp = ctx.enter_context(tc.tile_pool(name="hbp", bufs=1))
    sq2p = ctx.enter_context(tc.tile_pool(name="sq2p", bufs=2))
    wp = ctx.enter_context(tc.tile_pool(name="wp", bufs=1))
    outp = ctx.enter_context(tc.tile_pool(name="outp", bufs=8))
    h28p = ctx.enter_context(tc.tile_pool(name="h28p", bufs=4))
    g8p = ctx.enter_context(tc.tile_pool(name="g8p", bufs=8))
    yst = ctx.enter_context(tc.tile_pool(name="yst", bufs=3))
    rows = ctx.enter_context(tc.tile_pool(name="rows", bufs=2))
    r32 = ctx.enter_context(tc.tile_pool(name="r32", bufs=3))
    pmm = ctx.enter_context(tc.tile_pool(name="pmm", bufs=5, space="PSUM"))
    psm = ctx.enter_context(tc.tile_pool(name="psm", bufs=1, space="PSUM"))

    # ---- DMA queue: h_tok h0, KT0, h_tok h1, consts, hTb, KT1, pw, up8, dw8 ----
    htok_t = [htkp.tile([128, D], BF, tag="htok", name=f"htok{hf}")
              for hf in range(8)]
    nc.sync.dma_start(htok_t[0][:], a["h_tok"][0, :, :])
    nc.sync.dma_start(htok_t[1][:], a["h_tok"][1, :, :])
    kt0 = con.tile([128, 4, 512], BF, tag="kt0")
    nc.sync.dma_start(kt0[:], a["KT0p"][:, :, :])
    for hf in range(2, 8):
        nc.sync.dma_start(htok_t[hf][:], a["h_tok"][hf, :, :])

    def htok(ti):
        return htok_t[ti][:]

    # packed fp32 consts: eyef | projb | downb | upb
    cpf = con.tile([128, 128 + 2 * ND + NF], FP, tag="cpf")
    nc.sync.dma_start(cpf[:], a["cpf"][:, :])
    eyef = cpf[:, 0:128]
    projb = cpf[:, 128:128 + ND]
    downb = cpf[:, 128 + ND:128 + 2 * ND]
    upb = cpf[:, 128 + 2 * ND:128 + 2 * ND + NF]
    cpb = con.tile([128, ND * R], BF, tag="cpb")
    nc.sync.dma_start(cpb[:], a["cpb"][:, :])

    def v_sb(dj):
        return cpb[:, dj * R:(dj + 1) * R]

    gam_c = con.tile([R, 1], FP, tag="gam_c")
    nc.sync.dma_start(gam_c[:], a["gamma_t"][:, :])
    pu_sb = con.tile([R, D], BF, tag="pu_sb")
    nc.sync.dma_start(pu_sb[:], a["puT"][:, :])

    kt1 = con.tile([128, 8, 512], BF, tag="kt1")
    nc.sync.dma_start(kt1[:], a["KT1p"][:, :, :])

    # hTb [128, 8, W] bf16 (feature-major h: xv moving operand + residual)
    hbt = hbp.tile([128, ND, W], BF, tag="hb")
    nc.sync.dma_start(hbt[:], a["hTb"][:, :, :])

    def hb(dj):
        return hbt[:, dj, :]

    def kts(sj, tcc):
        return kt0[:, sj, :] if tcc == 0 else kt1[:, sj, :]

    pw_t = wp.tile([128, ND, D], BF, tag="pw")
    nc.sync.dma_start(pw_t[:], a["pw"][:, :, :])
    up_t = wp.tile([128, NF, 4, 2, 128], F8, tag="up8")
    nc.sync.dma_start(up_t[:], a["up8"][:, :, :, :, :])
    dw_t = wp.tile([128, ND, 8, 2, 128], F8, tag="dw8")
    nc.sync.dma_start(dw_t[:], a["dw8"][:, :, :, :, :])

    # ---- const-ap registrations (memsets, no DMA) ----
    zeros_c = con.tile([128, 1], FP, tag="zeros_c")
    nc.vector.memset(zeros_c[:], 0.0)
    nc.const_aps.aps[(FP, 0.0)] = zeros_c[:]
    eps_c = con.tile([128, 1], FP, tag="eps_c")
    nc.vector.memset(eps_c[:], 1e-8)
    nc.const_aps.aps[(FP, 1e-8)] = eps_c[:]
    ones_r128 = con.tile([1, 128], FP, tag="ones_r128")
    nc.vector.memset(ones_r128[:], 1.0)
    acc2 = con.tile([128, W], BF, tag="acc2")
    ones_rb = con.tile([1, 128], BF, tag="ones_rb")
    nc.vector.memset(ones_rb[:], 1.0)
    ones_cb = con.tile([128, 1], BF, tag="ones_cb")
    nc.vector.memset(ones_cb[:], 1.0)
    # pre-warm the ACT function tables while the first DMAs stream
    # (scratch target: std_col[:, 0:1] is overwritten later by the real Sqrt)

    # ---- rms1 stats (token-major ACT accum), two halves; hn in place ----
    ssq_col = con.tile([128, NT], FP, tag="ssq_col")
    std_col = con.tile([128, NT], FP, tag="std_col")
    rstd_col = con.tile([128, NT], FP, tag="rstd_col")
    nc.scalar.activation(std_col[:, 0:1], zeros_c[:], AF.Square)
    nc.scalar.activation(std_col[:, 0:1], zeros_c[:], AF.Sqrt, bias=1e-8, scale=1.0)
    nc.scalar.activation(std_col[:, 0:1], zeros_c[:], AF.Identity,
                         bias=eps_c[:, 0:1], scale=1.0)
    for ti in range(NT):
        nc.scalar.activation(acc2[:], htok(ti), AF.Square,
                             accum_out=ssq_col[:, ti:ti + 1])
        nc.scalar.activation(std_col[:, ti:ti + 1], ssq_col[:, ti:ti + 1],
                             AF.Sqrt, bias=1e-8, scale=1.0 / D)
        nc.vector.reciprocal(rstd_col[:, ti:ti + 1], std_col[:, ti:ti + 1])
        nc.vector.tensor_scalar_mul(htok(ti), htok(ti), rstd_col[:, ti:ti + 1])

    nc.vector.memset(acc2[:], 0.0)

    # ---- base mixing: out^T[d,t] = sum_s hn[s,d].T @ KT[s,t] ----
    outT = [outp.tile([128, W], BF, tag="outT", name=f"outT{dj}")
            for dj in range(ND)]

    def base_piece(tcc, lo, hi, sjs):
        # columns [lo, hi) of the tcc-chunk; causal s-blocks sjs
        w = hi - lo
        for dj in range(ND):
            po = pmm.tile([128, 512], FP, tag="pmm")
            for i, sj in enumerate(sjs):
                nc.tensor.matmul(po[:, 0:w],
                                 htok(sj)[:, dj * 128:(dj + 1) * 128],
                                 kts(sj, tcc)[:, lo:hi],
                                 start=(i == 0), stop=(i == len(sjs) - 1))
            nc.vector.tensor_copy(
                outT[dj][:, tcc * 512 + lo:tcc * 512 + hi], po[:, 0:w])

    base_piece(0, 0, 128, [0])
    base_piece(0, 128, 256, [0, 1])
    base_piece(0, 256, 512, [0, 1, 2, 3])

    # rstd_row [1, W] for the xv scale, via PE transposes of rstd_col
    prow = psm.tile([1, W], FP, tag="prow")
    for ti in range(NT):
        nc.tensor.transpose(prow[0:1, ti * 128:(ti + 1) * 128],
                            rstd_col[:, ti:ti + 1], eyef)
    rstd_row = rows.tile([1, W], FP, tag="row")
    nc.vector.tensor_copy(rstd_row[:], prow[:])

    base_piece(1, 0, 256, list(range(6)))
    base_piece(1, 256, 512, list(range(8)))

    # ---- xv^T [R, W] = v_eff.T @ h (raw), then * rstd ----
    xv_raw = r32.tile([R, W], FP, tag="r32")
    for c in range(2):
        pxv = psm.tile([R, 512], FP, tag="pxv", bufs=1)
        for dj in range(ND):
            nc.tensor.matmul(pxv[:], v_sb(dj), hb(dj)[:, c * 512:(c + 1) * 512],
                             start=(dj == 0), stop=(dj == ND - 1))
        nc.vector.tensor_copy(xv_raw[:, c * 512:(c + 1) * 512], pxv[:])
    rep32 = r32.tile([R, W], FP, tag="r32")
    for c in range(2):
        prep = psm.tile([R, 512], FP, tag="pxv", bufs=1)
        nc.tensor.matmul(prep[:], ones_r128[0:1, 0:R],
                         rstd_row[0:1, c * 512:(c + 1) * 512], start=True, stop=True)
        nc.vector.tensor_copy(rep32[:, c * 512:(c + 1) * 512], prep[:])
    xvT = r32.tile([R, W], FP, tag="r32")
    nc.vector.tensor_mul(xvT[:], xv_raw[:], rep32[:])

    # ---- decay scan (gamma broadcast along t); bf16 copy for the pu matmul ----
    mixedT = r32.tile([R, W], FP, tag="r32")
    nc.vector.tensor_tensor_scan(mixedT[:], gam_c[:].to_broadcast((R, W)), xvT[:],
                                 0.0, ALU.mult, ALU.add)
    mixedb = con.tile([R, W], BF, tag="mixedb")
    nc.vector.tensor_copy(mixedb[:], mixedT[:])

    # ---- h1^T = pw.T @ out^T + pu.T @ mixed^T + proj_b + h  (tcc-major);
    #      rms2 stats ride along: squares on GPSIMD, block-sums into acc2 (DVE),
    #      per-chunk rstd2/rep2/h28 overlap the other chunk's matmuls ----
    pssq2 = psm.tile([1, W], FP, tag="prow", bufs=1)
    h1T = [h1p.tile([128, W], FP, tag="h1", name=f"h1_{dj2}") for dj2 in range(ND)]
    h28 = [h28p.tile([128, 2, W], F8, tag="h28", name=f"h28_{m}") for m in range(4)]
    std2 = rows.tile([1, W], FP, tag="row")
    rstd2_row = rows.tile([1, W], FP, tag="row")
    rstd2b = con.tile([1, W], BF, tag="rstd2b")
    rep2 = con.tile([128, W], FP, tag="rep2")

    def rms2_reduce(c):
        sl = slice(c * 512, (c + 1) * 512)
        nc.tensor.matmul(pssq2[0:1, sl], ones_cb[:], acc2[:, sl],
                         start=True, stop=True)
        nc.scalar.activation(std2[0:1, sl], pssq2[0:1, sl], AF.Sqrt,
                             bias=1e-8, scale=1.0 / D)
        nc.vector.reciprocal(rstd2_row[0:1, sl], std2[0:1, sl])
        nc.vector.tensor_copy(rstd2b[0:1, sl], rstd2_row[0:1, sl])

    def rep2_h28(c):
        sl = slice(c * 512, (c + 1) * 512)
        pr2 = pmm.tile([128, 512], FP, tag="pmm")
        nc.tensor.matmul(pr2[:], ones_rb[:], rstd2b[0:1, sl],
                         start=True, stop=True)
        nc.vector.tensor_copy(rep2[:, sl], pr2[:])
        for m in range(4):
            for i in range(2):
                eng = nc.vector if (2 * m + i) % 2 == 0 else nc.gpsimd
                eng.tensor_mul(h28[m][:, i, sl], h1T[2 * m + i][:, sl],
                               rep2[:, sl])

    for tcc in range(2):
        sl = slice(tcc * 512, (tcc + 1) * 512)
        for dj2 in range(ND):
            if tcc == 1 and dj2 == 2:
                rms2_reduce(0)
            if tcc == 1 and dj2 == 5:
                rep2_h28(0)
            ph = pmm.tile([128, 512], FP, tag="pmm")
            for dj in range(ND):
                nc.tensor.matmul(ph[:], pw_t[:, dj2, dj * 128:(dj + 1) * 128],
                                 outT[dj][:, sl],
                                 start=(dj == 0), stop=False)
            nc.tensor.matmul(ph[:], pu_sb[:, dj2 * 128:(dj2 + 1) * 128],
                             mixedb[:, sl], start=False, stop=True)
            h1 = h1T[dj2]
            nc.scalar.activation(h1[:, sl], ph[:], AF.Identity,
                                 bias=projb[:, dj2:dj2 + 1], scale=1.0)
            nc.vector.tensor_add(h1[:, sl], h1[:, sl], hb(dj2)[:, sl])
            s2 = sq2p.tile([128, 512], BF, tag="sq2")
            nc.gpsimd.tensor_mul(s2[:], h1[:, sl], h1[:, sl])
            nc.vector.tensor_add(acc2[:, sl], acc2[:, sl], s2[:])

    # ---- up + gelu then down + residual, tcc-major (fp8 DoubleRow) ----
    g8 = [g8p.tile([128, 2, W], F8, tag="g8", name=f"g8_{m}") for m in range(NF // 2)]
    for tcc in range(2):
        sl = slice(tcc * 512, (tcc + 1) * 512)
        for fi in range(NF):
            pg = pmm.tile([128, 512], FP, tag="pmm")
            for m in range(4):
                nc.tensor.matmul(pg[:], up_t[:, fi, m, :, :], h28[m][:, :, sl],
                                 start=(m == 0), stop=(m == 3),
                                 perf_mode=PM.DoubleRow)
            nc.scalar.activation(g8[fi // 2][:, fi % 2, sl], pg[:],
                                 AF.Gelu_apprx_tanh,
                                 bias=upb[:, fi:fi + 1], scale=1.0 / UP_SCALE)
        for dj2 in range(ND):
            if tcc == 0 and dj2 == 1:
                rms2_reduce(1)
            if tcc == 0 and dj2 == 3:
                rep2_h28(1)
            py = pmm.tile([128, 512], FP, tag="pmm")
            for m in range(8):
                nc.tensor.matmul(py[:], dw_t[:, dj2, m, :, :], g8[m][:, :, sl],
                                 start=(m == 0), stop=(m == 7),
                                 perf_mode=PM.DoubleRow)
            y = yst.tile([128, 512], FP, tag="yst")
            nc.scalar.activation(y[:], py[:], AF.Identity,
                                 bias=downb[:, dj2:dj2 + 1], scale=1.0 / DW_SCALE)
            nc.vector.tensor_add(y[:], y[:], h1T[dj2][:, sl])
            nc.sync.dma_start(a["yT"][dj2, :, sl], y[:])


_NC_CACHE = {}


def _build():
    if "nc" in _NC_CACHE:
        return _NC_CACHE["nc"]
    nc = bacc.Bacc("TRN2", target_bir_lowering=False, debug=False)

    def P(name, shape, dt=FP, out=False):
        return nc.declare_dram_parameter(name, list(shape), dt, isOutput=out)

    a = dict(
        h_tok=P("h_tok", (NT, 128, D), BF),
        hTb=P("hTb", (128, ND, W), BF),
        KT0p=P("KT0p", (128, 4, 512), BF),
        KT1p=P("KT1p", (128, 8, 512), BF),
        cpf=P("cpf", (128, 128 + 2 * ND + NF)),
        cpb=P("cpb", (128, ND * R), BF),
        puT=P("puT", (R, D), BF),
        pw=P("pw", (128, ND, D), BF),
        up8=P("up8", (128, NF, 4, 2, 128), F8),
        dw8=P("dw8", (128, ND, 8, 2, 128), F8),
        gamma_t=P("gamma_t", (R, 1)),
        yT=P("yT", (ND, 128, W), out=True),
    )
    with ExitStack() as ctx:
        tcx = ctx.enter_context(tile.TileContext(nc))
        _emit(ctx, tcx, a)
    nc.finalize()
    _NC_CACHE["nc"] = nc
    return nc


def _sigmoid(x):
    return 1.0 / (1.0 + np.exp(-x))


def host_prep(inputs):
    """Exact host-side weight folds/layout. Returns the shared in_map dict."""
    f32 = np.float32
    ns1 = np.asarray(inputs["norm1_scale"], f32)
    ns2 = np.asarray(inputs["norm2_scale"], f32)
    gate = f32(_sigmoid(np.float64(np.asarray(inputs["gate_logit"]))))
    alpha = f32(_sigmoid(np.float64(np.asarray(inputs["alpha_logit"]))))
    gamma = (GAMMA_MIN + (GAMMA_MAX - GAMMA_MIN)
             * _sigmoid(np.asarray(inputs["decay_logit"], np.float64))).astype(f32)

    kb = np.asarray(inputs["k_base"], f32) * np.tril(np.ones((W, W), f32))
    KT = np.ascontiguousarray((gate * kb).T).astype(BFNP)
    KT0p = np.stack([KT[sj * 128:(sj + 1) * 128, 0:512] for sj in range(4)], axis=1)
    KT1p = np.stack([KT[sj * 128:(sj + 1) * 128, 512:1024] for sj in range(8)], axis=1)
    v_eff = (ns1[:, None] * np.asarray(inputs["v"], f32)).astype(BFNP)
    cpb = np.ascontiguousarray(
        v_eff.reshape(ND, 128, R).transpose(1, 0, 2).reshape(128, ND * R))
    proj_w = np.asarray(inputs["proj_w"], f32)
    puT = np.ascontiguousarray(
        (alpha * (proj_w @ np.asarray(inputs["u"], f32))).T).astype(BFNP)
    pw_lhsT = (proj_w * ns1[None, :]).T
    up_lhsT = (np.asarray(inputs["up_w"], f32) * ns2[None, :]).T
    dw_lhsT = np.asarray(inputs["down_w"], f32).T

    # stationary-block layouts, contraction-sub-128 on the partition axis
    pw = np.ascontiguousarray(
        pw_lhsT.reshape(ND, 128, ND, 128).transpose(1, 2, 0, 3).reshape(128, ND, D)
    ).astype(BFNP)
    up8 = np.ascontiguousarray(
        (UP_SCALE * up_lhsT).reshape(4, 2, 128, NF, 128).transpose(2, 3, 0, 1, 4)
    ).astype(F8NP)
    dw8 = np.ascontiguousarray(
        (DW_SCALE * dw_lhsT).reshape(8, 2, 128, ND, 128).transpose(2, 3, 0, 1, 4)
    ).astype(F8NP)

    cpf = np.zeros((128, 128 + 2 * ND + NF), f32)
    cpf[:, 0:128] = np.eye(128, dtype=f32)
    cpf[:, 128:128 + ND] = np.asarray(inputs["proj_b"], f32).reshape(ND, 128).T
    cpf[:, 128 + ND:128 + 2 * ND] = (
        np.asarray(inputs["down_b"], f32).reshape(ND, 128).T)
    cpf[:, 128 + 2 * ND:] = np.asarray(inputs["up_b"], f32).reshape(NF, 128).T

    return dict(
        KT0p=KT0p, KT1p=KT1p, cpf=cpf, cpb=cpb, puT=puT, pw=pw, up8=up8, dw8=dw8,
        gamma_t=np.ascontiguousarray(gamma[:, None]),
    )


def make_in_maps(inputs):
    shared = host_prep(inputs)
    h = np.asarray(inputs["h"], np.float32)
    in_maps = []
    for b in range(B):
        m = dict(shared)
        hb16 = h[b].astype(BFNP)
        m["h_tok"] = np.ascontiguousarray(hb16.reshape(NT, 128, D))
        m["hTb"] = np.ascontiguousarray(
            hb16.T.reshape(ND, 128, W).transpose(1, 0, 2))
        in_maps.append(m)
    return in_maps


def kernel(**inputs):
    nc = _build()
    in_maps = make_in_maps(inputs)
    res = run_bass_kernel_spmd(nc, in_maps, list(range(B)))
    out = np.stack([np.asarray(res.results[i]["yT"]).reshape(D, W).T
                    for i in range(B)])
    return np.ascontiguousarray(out.astype(np.float32))
